# Initial kernel scaffold
#
"""Trainium2 Bass kernel for a dense transformer block (attention + FFN, 2x LN).

Sharding (8 cores): tensor-parallel over heads for attention (each core owns
H/8 = 2 heads for BOTH batch elements), followed by an AllToAll that
redistributes the per-head attention outputs into per-row shards, after which
each core runs proj + LN1 + FFN + LN2 for its own B*T/8 = 512 rows.
No K/V recompute, single collective, balanced compute.

Matmul inputs are bf16 (fp32 PSUM accumulation); LayerNorm math is fp32.
"""

import sys
import types

sys.path.insert(0, "/opt/trn_rl_repo")
sys.path.insert(0, "/root/.axon_site")

import numpy as np
import ml_dtypes

import concourse.bass as bass
import concourse.mybir as mybir
import concourse.tile as tile
from concourse import bacc
from concourse.masks import make_identity, make_causal_mask

BF16 = mybir.dt.bfloat16
F32 = mybir.dt.float32
EPS = 1e-5


def _bcast_ap(ap, parts=128):
    """Partition-broadcast a DRAM AP: [d...] -> [parts, d...] with step 0."""
    return bass.AP(tensor=ap.tensor, offset=ap.offset, ap=[[0, parts]] + list(ap.ap))


def build(B=2, T=2048, C=1024, H=16, FF=4096, n_cores=8):
    """Build the SPMD Bass module. Returns (nc, cfg)."""
    HS = C // H
    HPC = H // n_cores          # heads per core
    D2 = HPC * HS               # local head channels (partition dim of QT/KT)
    S = B * T // n_cores        # output rows per core
    CC = C // 128               # contraction chunks
    FFC = FF // 128
    G_PB = n_cores // B         # row-groups per batch
    QG = S // 128               # 128-row q-tiles per group
    NT = T // 128               # 128-row tiles per batch
    SC = 512                    # score free-dim chunk
    scale = float(HS) ** -0.5

    assert S % 128 == 0 and C % 128 == 0 and T % 512 == 0 and D2 <= 128
    assert HS in (32, 64)  # partition offsets must be 32-aligned

    nc = bacc.Bacc("TRN2", target_bir_lowering=False, debug=False,
                   num_devices=n_cores)

    # ---- I/O ----
    x_d = nc.dram_tensor("x", [B * T, C], BF16, kind="ExternalInput").ap()
    wq_d = nc.dram_tensor("wq", [C, D2], BF16, kind="ExternalInput").ap()
    wk_d = nc.dram_tensor("wk", [C, D2], BF16, kind="ExternalInput").ap()
    wv_d = nc.dram_tensor("wv", [C, D2], BF16, kind="ExternalInput").ap()
    bq_d = nc.dram_tensor("bq", [D2, 1], F32, kind="ExternalInput").ap()
    bk_d = nc.dram_tensor("bk", [D2, 1], F32, kind="ExternalInput").ap()
    bv_d = nc.dram_tensor("bv", [D2], F32, kind="ExternalInput").ap()
    wp_d = nc.dram_tensor("wp", [C, C], BF16, kind="ExternalInput").ap()
    bp_d = nc.dram_tensor("bp", [C], F32, kind="ExternalInput").ap()
    w1_d = nc.dram_tensor("w1t", [FFC, C, 128], BF16, kind="ExternalInput").ap()
    b1_d = nc.dram_tensor("b1", [FF, 1], F32, kind="ExternalInput").ap()
    w2_d = nc.dram_tensor("w2", [FF, C], BF16, kind="ExternalInput").ap()
    b2_d = nc.dram_tensor("b2", [C], F32, kind="ExternalInput").ap()
    g1_d = nc.dram_tensor("g1", [C], F32, kind="ExternalInput").ap()
    be1_d = nc.dram_tensor("be1", [C], F32, kind="ExternalInput").ap()
    g2_d = nc.dram_tensor("g2", [C], F32, kind="ExternalInput").ap()
    be2_d = nc.dram_tensor("be2", [C], F32, kind="ExternalInput").ap()
    y_d = nc.dram_tensor("y", [S, C], F32, kind="ExternalOutput").ap()

    with tile.TileContext(nc) as tc:
        import contextlib
        ctx = contextlib.ExitStack()
        with ctx:
            consts = ctx.enter_context(tc.tile_pool(name="consts", bufs=1))
            persist = ctx.enter_context(tc.tile_pool(name="persist", bufs=1))
            work = ctx.enter_context(tc.tile_pool(name="work", bufs=3))
            psum = ctx.enter_context(tc.tile_pool(name="psum", bufs=2, space="PSUM"))
            dram = ctx.enter_context(tc.tile_pool(name="dram", bufs=1, space="DRAM"))

            # ---- constants ----
            ident = consts.tile([128, 128], BF16)
            make_identity(nc, ident)
            cmask = consts.tile([128, 128], F32)
            make_causal_mask(nc, cmask, mask_val=-1e10)
            eps_t = consts.tile([128, 1], F32)
            nc.vector.memset(eps_t, EPS)

            bq_s = consts.tile([D2, 1], F32, tag="bq")
            nc.sync.dma_start(out=bq_s, in_=bq_d)
            bk_s = consts.tile([D2, 1], F32, tag="bk")
            nc.sync.dma_start(out=bk_s, in_=bk_d)
            bv_s = consts.tile([128, D2], F32, tag="bv")
            nc.sync.dma_start(out=bv_s, in_=_bcast_ap(bv_d))
            bp_s = consts.tile([128, C], F32, tag="bp")
            nc.sync.dma_start(out=bp_s, in_=_bcast_ap(bp_d))
            b2_s = consts.tile([128, C], F32, tag="b2")
            nc.sync.dma_start(out=b2_s, in_=_bcast_ap(b2_d))
            g1_s = consts.tile([128, C], F32, tag="g1")
            nc.sync.dma_start(out=g1_s, in_=_bcast_ap(g1_d))
            be1_s = consts.tile([128, C], F32, tag="be1")
            nc.sync.dma_start(out=be1_s, in_=_bcast_ap(be1_d))
            g2_s = consts.tile([128, C], F32, tag="g2")
            nc.sync.dma_start(out=g2_s, in_=_bcast_ap(g2_d))
            be2_s = consts.tile([128, C], F32, tag="be2")
            nc.sync.dma_start(out=be2_s, in_=_bcast_ap(be2_d))
            b1_s = consts.tile([128, FFC, 1], F32, tag="b1")
            nc.sync.dma_start(out=b1_s, in_=b1_d.rearrange("(fh fl) o -> fl fh o", fl=128))

            # qkv weights: [C, D2] -> sbuf [128, CC, D2]
            wq_s = consts.tile([128, CC, D2], BF16, tag="wq")
            nc.sync.dma_start(out=wq_s, in_=wq_d.rearrange("(ch cl) d -> cl ch d", cl=128))
            wk_s = consts.tile([128, CC, D2], BF16, tag="wk")
            nc.sync.dma_start(out=wk_s, in_=wk_d.rearrange("(ch cl) d -> cl ch d", cl=128))
            wv_s = consts.tile([128, CC, D2], BF16, tag="wv")
            nc.sync.dma_start(out=wv_s, in_=wv_d.rearrange("(ch cl) d -> cl ch d", cl=128))
            wp_s = consts.tile([128, CC, C], BF16, tag="wp")
            nc.sync.dma_start(out=wp_s, in_=wp_d.rearrange("(ch cl) e -> cl ch e", cl=128))

            # ---- persistent activations ----
            xT = [persist.tile([128, CC, T], BF16, tag=f"xT{b}") for b in range(B)]
            QT = [persist.tile([D2, T], BF16, tag=f"QT{b}") for b in range(B)]
            KT = [persist.tile([D2, T], BF16, tag=f"KT{b}") for b in range(B)]
            V = [persist.tile([128, NT, D2], BF16, tag=f"V{b}") for b in range(B)]

            # ---- phase A: load x, transpose to xT ----
            for b in range(B):
                for i in range(NT):
                    xt = work.tile([128, C], BF16, tag="x_in")
                    nc.sync.dma_start(out=xt, in_=x_d[b * T + i * 128: b * T + (i + 1) * 128, :])
                    for k in range(CC):
                        pt = psum.tile([128, 128], BF16, tag="tp", bufs=3)
                        nc.tensor.transpose(pt, xt[:, k * 128:(k + 1) * 128], ident)
                        nc.vector.tensor_copy(out=xT[b][:, k, i * 128:(i + 1) * 128], in_=pt)

            # ---- phase B: Q/K projections ([D2, T]) and V ([s, D2]) ----
            for b in range(B):
                for w_s, bias_s, dst in ((wq_s, bq_s, QT[b]), (wk_s, bk_s, KT[b])):
                    for j in range(T // SC):
                        ps = psum.tile([D2, SC], F32, tag="qk_ps", bufs=3)
                        for k in range(CC):
                            nc.tensor.matmul(ps, w_s[:, k, :], xT[b][:, k, j * SC:(j + 1) * SC],
                                             start=(k == 0), stop=(k == CC - 1))
                        nc.vector.tensor_scalar_add(out=dst[:, j * SC:(j + 1) * SC],
                                                    in0=ps, scalar1=bias_s)
                for sb in range(NT):
                    ps = psum.tile([128, D2], F32, tag="v_ps", bufs=3)
                    for k in range(CC):
                        nc.tensor.matmul(ps, xT[b][:, k, sb * 128:(sb + 1) * 128], wv_s[:, k, :],
                                         start=(k == 0), stop=(k == CC - 1))
                    nc.vector.tensor_tensor(out=V[b][:, sb, :], in0=ps, in1=bv_s,
                                            op=mybir.AluOpType.add)

            # ---- phase C: attention per (batch, row-group, head) ----
            for b in range(B):
                for g in range(G_PB):
                    n_sb = (g + 1) * QG  # s-blocks this group attends to
                    outT_s = work.tile([D2, S], BF16, tag="outT", bufs=2)
                    for h in range(HPC):
                        hofs = h * HS
                        for ql in range(QG):
                            qi = g * QG + ql
                            s_len = (qi + 1) * 128
                            n_ch = (s_len + SC - 1) // SC
                            p_t = work.tile([128, n_sb * 128], BF16, tag="p", bufs=2)
                            sums = work.tile([128, (n_sb * 128 + SC - 1) // SC], F32,
                                             tag="sums", bufs=2)
                            for ci in range(n_ch):
                                w = min(SC, s_len - ci * SC)
                                ps = psum.tile([128, SC], F32, tag="sc", bufs=3)
                                nc.tensor.matmul(
                                    ps[:, :w],
                                    QT[b][hofs:hofs + HS, qi * 128:(qi + 1) * 128],
                                    KT[b][hofs:hofs + HS, ci * SC:ci * SC + w],
                                    start=True, stop=True)
                                if (ci + 1) * SC >= s_len:  # chunk w/ diagonal block
                                    nc.vector.tensor_tensor(
                                        out=ps[:, w - 128:w], in0=ps[:, w - 128:w],
                                        in1=cmask, op=mybir.AluOpType.add)
                                nc.scalar.activation(
                                    out=p_t[:, ci * SC:ci * SC + w], in_=ps[:, :w],
                                    func=mybir.ActivationFunctionType.Exp,
                                    scale=scale, accum_out=sums[:, ci:ci + 1])
                            tot = work.tile([128, 1], F32, tag="tot", bufs=2)
                            nc.vector.reduce_sum(tot, sums[:, :n_ch], mybir.AxisListType.X)
                            rec = work.tile([128, 1], F32, tag="rec", bufs=2)
                            nc.vector.reciprocal(rec, tot)
                            nc.vector.tensor_scalar_mul(out=p_t[:, :s_len],
                                                        in0=p_t[:, :s_len], scalar1=rec)
                            # transpose P -> PT[sb] column block ql
                            for sb in range(qi + 1):
                                pt_ps = psum.tile([128, 128], BF16, tag="ptp", bufs=3)
                                nc.tensor.transpose(pt_ps, p_t[:, sb * 128:(sb + 1) * 128], ident)
                                ptt = _pt_tile(tc, persist, sb, S)
                                nc.vector.tensor_copy(out=ptt[:, ql * 128:(ql + 1) * 128],
                                                      in_=pt_ps)
                        # attnV for head h
                        avp = psum.tile([HS, S], F32, tag="av", bufs=2)
                        for sb in range(n_sb):
                            sb_l = sb - g * QG
                            col0 = 0 if sb_l < 0 else sb_l * 128
                            ptt = _pt_tile(tc, persist, sb, S)
                            nc.tensor.matmul(avp[:, col0:], V[b][:, sb, hofs:hofs + HS],
                                             ptt[:, col0:],
                                             start=(sb == 0), stop=(sb == n_sb - 1))
                        nc.vector.tensor_copy(out=outT_s[hofs:hofs + HS, :], in_=avp)
                    j = b * G_PB + g
                    a2a_in = _a2a_in(tc, dram, C, S)
                    nc.sync.dma_start(out=a2a_in[j * D2:(j + 1) * D2, :], in_=outT_s)

            # ---- phase D: AllToAll ----
            a2a_in = _a2a_in(tc, dram, C, S)
            a2a_out = dram.tile([C, S], BF16, tag="a2a_out")
            nc.gpsimd.collective_compute(
                "AllToAll", mybir.AluOpType.bypass,
                replica_groups=[list(range(n_cores))],
                ins=[a2a_in.opt()], outs=[a2a_out.opt()])

            # ---- phase E: proj + bp + LN1 ----
            ot_s = persist.tile([128, CC, S], BF16, tag="ot")
            nc.sync.dma_start(out=ot_s, in_=a2a_out.rearrange("(ch cl) s -> cl ch s", cl=128))
            hln = persist.tile([128, S // 128, C], BF16, tag="hln")
            for ti in range(S // 128):
                hps = []
                for e in range(C // SC):
                    pp = psum.tile([128, SC], F32, tag=f"proj{e}", bufs=2)
                    for k in range(CC):
                        nc.tensor.matmul(pp, ot_s[:, k, ti * 128:(ti + 1) * 128],
                                         wp_s[:, k, e * SC:(e + 1) * SC],
                                         start=(k == 0), stop=(k == CC - 1))
                    hps.append(pp)
                h_t = work.tile([128, C], F32, tag="h", bufs=2)
                for e, pp in enumerate(hps):
                    nc.vector.tensor_tensor(out=h_t[:, e * SC:(e + 1) * SC], in0=pp,
                                            in1=bp_s[:, e * SC:(e + 1) * SC],
                                            op=mybir.AluOpType.add)
                _ln(nc, work, h_t, g1_s, be1_s, hln[:, ti, :], eps_t, C)

            # ---- phase F: transpose hln -> hlnT [128, CC, S] ----
            hlnT = persist.tile([128, CC, S], BF16, tag="hlnT")
            for ti in range(S // 128):
                for k in range(CC):
                    pt = psum.tile([128, 128], BF16, tag="tp2", bufs=3)
                    nc.tensor.transpose(pt, hln[:, ti, k * 128:(k + 1) * 128], ident)
                    nc.vector.tensor_copy(out=hlnT[:, k, ti * 128:(ti + 1) * 128], in_=pt)

            # ---- phase G: FFN1 (fT = relu(W1.T @ hlnT + b1)) ----
            fT = persist.tile([128, FFC, S], BF16, tag="fT")
            for f in range(FFC):
                w1_t = work.tile([128, CC, 128], BF16, tag="w1", bufs=3)
                nc.sync.dma_start(out=w1_t, in_=w1_d[f].rearrange("(ch cl) f -> cl ch f", cl=128))
                fp = psum.tile([128, S], F32, tag="f1", bufs=2)
                for k in range(CC):
                    nc.tensor.matmul(fp, w1_t[:, k, :], hlnT[:, k, :],
                                     start=(k == 0), stop=(k == CC - 1))
                nc.scalar.activation(out=fT[:, f, :], in_=fp,
                                     func=mybir.ActivationFunctionType.Relu,
                                     bias=b1_s[:, f, :])

            # ---- phase H: FFN2 (g = relu(fT.T @ W2 + b2)), + LN2 -> y ----
            n_eh = C // SC
            gps = [[psum.tile([128, SC], F32, tag=f"g{t}_{e}", bufs=1)
                    for e in range(n_eh)] for t in range(S // 128)]
            for f in range(FFC):
                w2_t = work.tile([128, C], BF16, tag="w2", bufs=3)
                nc.sync.dma_start(out=w2_t, in_=w2_d[f * 128:(f + 1) * 128, :])
                for t in range(S // 128):
                    for e in range(n_eh):
                        nc.tensor.matmul(gps[t][e], fT[:, f, t * 128:(t + 1) * 128],
                                         w2_t[:, e * SC:(e + 1) * SC],
                                         start=(f == 0), stop=(f == FFC - 1))
            for t in range(S // 128):
                g_t = work.tile([128, C], F32, tag="gt", bufs=2)
                for e in range(n_eh):
                    nc.vector.tensor_tensor(out=g_t[:, e * SC:(e + 1) * SC], in0=gps[t][e],
                                            in1=b2_s[:, e * SC:(e + 1) * SC],
                                            op=mybir.AluOpType.add)
                nc.vector.tensor_scalar_max(out=g_t, in0=g_t, scalar1=0.0)
                o_t = work.tile([128, C], F32, tag="ot_f", bufs=2)
                _ln(nc, work, g_t, g2_s, be2_s, o_t, eps_t, C)
                nc.sync.dma_start(out=y_d[t * 128:(t + 1) * 128, :], in_=o_t)

    nc.compile()
    cfg = dict(B=B, T=T, C=C, H=H, HS=HS, HPC=HPC, D2=D2, S=S, FF=FF,
               n_cores=n_cores, G_PB=G_PB)
    return nc, cfg


_PT_CACHE = {}


def _pt_tile(tc, pool, sb, S):
    key = (id(tc), sb)
    if key not in _PT_CACHE:
        _PT_CACHE[key] = pool.tile([128, S], BF16, tag=f"pt{sb}")
    return _PT_CACHE[key]


_A2A_CACHE = {}


def _a2a_in(tc, pool, C, S):
    key = id(tc)
    if key not in _A2A_CACHE:
        _A2A_CACHE[key] = pool.tile([C, S], BF16, tag="a2a_in")
    return _A2A_CACHE[key]


def _ln(nc, pool, x_t, gamma_s, beta_s, out_ap, eps_t, C):
    """LayerNorm rows of x_t [128, C] f32 -> out_ap (any dtype)."""
    ng = (C + 511) // 512
    stats = pool.tile([128, ng, 6], F32, tag="ln_stats", bufs=2)
    for i in range(ng):
        w = min(512, C - i * 512)
        nc.vector.bn_stats(out=stats[:, i, :], in_=x_t[:, i * 512:i * 512 + w])
    mv = pool.tile([128, 2], F32, tag="ln_mv", bufs=2)
    nc.vector.bn_aggr(out=mv, in_=stats)
    rstd = pool.tile([128, 1], F32, tag="ln_rstd", bufs=2)
    nc.scalar.activation(out=rstd, in_=mv[:, 1:2],
                         func=mybir.ActivationFunctionType.Sqrt, bias=eps_t)
    nc.vector.reciprocal(rstd, rstd)
    tmp = pool.tile([128, C], F32, tag="ln_tmp", bufs=2)
    nc.vector.tensor_scalar(out=tmp, in0=x_t, scalar1=mv[:, 0:1], scalar2=rstd,
                            op0=mybir.AluOpType.subtract, op1=mybir.AluOpType.mult)
    nc.vector.tensor_tensor(out=tmp, in0=tmp, in1=gamma_s, op=mybir.AluOpType.mult)
    nc.vector.tensor_tensor(out=out_ap, in0=tmp, in1=beta_s, op=mybir.AluOpType.add)


# ============================================================================
# Host side: shard inputs, run SPMD, unshard output.
# ============================================================================

def make_in_maps(inputs, cfg):
    """Build the per-core input dicts from full-size numpy inputs."""
    B, T, C, H = cfg["B"], cfg["T"], cfg["C"], cfg["H"]
    HS, HPC, FF, n_cores = cfg["HS"], cfg["HPC"], cfg["FF"], cfg["n_cores"]
    FFC = FF // 128
    bf = ml_dtypes.bfloat16

    f = {k: np.asarray(v, np.float32) for k, v in inputs.items()}
    x_flat = f["x"].reshape(B * T, C).astype(bf)
    w1t = f["W1"].reshape(C, FFC, 128).transpose(1, 0, 2).astype(bf)
    w2 = f["W2"].astype(bf)
    wp = f["Wp"].astype(bf)
    shared = dict(
        x=x_flat, wp=wp, bp=f["bp"], w1t=w1t, b1=f["b1"].reshape(FF, 1),
        w2=w2, b2=f["b2"], g1=f["g1"], be1=f["be1"], g2=f["g2"], be2=f["be2"],
    )
    in_maps = []
    for c in range(n_cores):
        hs = slice(c * HPC, (c + 1) * HPC)
        # [HPC, C, HS] -> [C, HPC*HS]
        wq = f["Wq"][hs].transpose(1, 0, 2).reshape(C, HPC * HS).astype(bf)
        wk = f["Wk"][hs].transpose(1, 0, 2).reshape(C, HPC * HS).astype(bf)
        wv = f["Wv"][hs].transpose(1, 0, 2).reshape(C, HPC * HS).astype(bf)
        bq = f["bq"][hs].reshape(HPC * HS, 1)
        bk = f["bk"][hs].reshape(HPC * HS, 1)
        bv = f["bv"][hs].reshape(HPC * HS)
        in_maps.append(dict(shared, wq=wq, wk=wk, wv=wv, bq=bq, bk=bk, bv=bv))
    return in_maps


def unshard(results, cfg):
    B, T, C, S = cfg["B"], cfg["T"], cfg["C"], cfg["S"]
    out = np.empty((B * T, C), np.float32)
    for c, r in enumerate(results):
        out[c * S:(c + 1) * S, :] = r["y"]
    return out.reshape(B, T, C)


_BUILT = None


def _get_built():
    global _BUILT
    if _BUILT is None:
        _BUILT = build()
    return _BUILT


def _install_ntff_hook():
    """Best-effort: register the axon NTFF profiling hook (for trace=True)."""
    try:
        import antenv
        if "antenv.axon_hooks" not in sys.modules:
            m = types.ModuleType("antenv.axon_hooks")
            h = [None]
            m.set_axon_ntff_profile_hook = lambda x: h.__setitem__(0, x)
            m.get_axon_ntff_profile_hook = lambda: h[0]
            sys.modules["antenv.axon_hooks"] = m
            antenv.axon_hooks = m
            from trn_agent_boot.trn_boot import _ntff_profile_via_ctypes
            m.set_axon_ntff_profile_hook(
                _ntff_profile_via_ctypes("/opt/axon/libaxon_pjrt.so"))
    except Exception:
        pass


def run(inputs, trace=False):
    _install_ntff_hook()
    from concourse import bass_utils
    bass_utils.upload_artifacts = lambda tmpdir: "local://" + tmpdir
    nc, cfg = _get_built()
    in_maps = make_in_maps(inputs, cfg)
    res = bass_utils.run_bass_kernel_spmd(
        nc, in_maps, list(range(cfg["n_cores"])), trace=trace)
    return unshard(res.results, cfg), res


def kernel(**inputs):
    out, _ = run(inputs, trace=False)
    return out


# revision 3
# speedup vs baseline: 2.7888x; 2.7888x over previous
"""Trainium2 Bass kernel for a dense transformer block (attention + FFN, 2x LN).

Sharding (8 cores): tensor-parallel over heads for attention (each core owns
H/8 = 2 heads for BOTH batch elements), followed by an AllToAll that
redistributes the per-head attention outputs into per-row shards, after which
each core runs proj + LN1 + FFN + LN2 for its own B*T/8 = 512 rows.
No K/V recompute, single collective, balanced compute.

Matmul inputs are bf16 (fp32 PSUM accumulation); LayerNorm math is fp32.
"""

import sys
import types

sys.path.insert(0, "/opt/trn_rl_repo")
sys.path.insert(0, "/root/.axon_site")

import numpy as np
import ml_dtypes

import concourse.bass as bass
import concourse.mybir as mybir
import concourse.tile as tile
from concourse import bacc
from concourse.masks import make_identity, make_causal_mask

BF16 = mybir.dt.bfloat16
F32 = mybir.dt.float32
EPS = 1e-5


def _bcast_ap(ap, parts=128):
    """Partition-broadcast a DRAM AP: [d...] -> [parts, d...] with step 0."""
    return bass.AP(tensor=ap.tensor, offset=ap.offset, ap=[[0, parts]] + list(ap.ap))


def build(B=2, T=2048, C=1024, H=16, FF=4096, n_cores=8):
    """Build the SPMD Bass module. Returns (nc, cfg)."""
    HS = C // H
    HPC = H // n_cores          # heads per core
    D2 = HPC * HS               # local head channels (partition dim of QT/KT)
    S = B * T // n_cores        # output rows per core
    CC = C // 128               # contraction chunks
    FFC = FF // 128
    G_PB = n_cores // B         # row-groups per batch
    QG = S // 128               # 128-row q-tiles per group
    NT = T // 128               # 128-row tiles per batch
    SC = 512                    # score free-dim chunk
    scale = float(HS) ** -0.5

    assert S % 128 == 0 and C % 128 == 0 and T % 512 == 0 and D2 <= 128
    assert HS in (32, 64)  # partition offsets must be 32-aligned

    nc = bacc.Bacc("TRN2", target_bir_lowering=False, debug=False,
                   num_devices=n_cores)

    # ---- I/O ----
    x_d = nc.dram_tensor("x", [B * T, C], BF16, kind="ExternalInput").ap()
    wq_d = nc.dram_tensor("wq", [C, D2], BF16, kind="ExternalInput").ap()
    wk_d = nc.dram_tensor("wk", [C, D2], BF16, kind="ExternalInput").ap()
    wv_d = nc.dram_tensor("wv", [C, D2], BF16, kind="ExternalInput").ap()
    bq_d = nc.dram_tensor("bq", [D2, 1], F32, kind="ExternalInput").ap()
    bk_d = nc.dram_tensor("bk", [D2, 1], F32, kind="ExternalInput").ap()
    bv_d = nc.dram_tensor("bv", [D2], F32, kind="ExternalInput").ap()
    wp_d = nc.dram_tensor("wp", [C, C], BF16, kind="ExternalInput").ap()
    bp_d = nc.dram_tensor("bp", [C], F32, kind="ExternalInput").ap()
    w1_d = nc.dram_tensor("w1t", [FFC, C, 128], BF16, kind="ExternalInput").ap()
    b1_d = nc.dram_tensor("b1", [FF, 1], F32, kind="ExternalInput").ap()
    w2_d = nc.dram_tensor("w2", [FF, C], BF16, kind="ExternalInput").ap()
    b2_d = nc.dram_tensor("b2", [C], F32, kind="ExternalInput").ap()
    g1_d = nc.dram_tensor("g1", [C], F32, kind="ExternalInput").ap()
    be1_d = nc.dram_tensor("be1", [C], F32, kind="ExternalInput").ap()
    g2_d = nc.dram_tensor("g2", [C], F32, kind="ExternalInput").ap()
    be2_d = nc.dram_tensor("be2", [C], F32, kind="ExternalInput").ap()
    y_d = nc.dram_tensor("y", [S, C], F32, kind="ExternalOutput").ap()

    with tile.TileContext(nc) as tc:
        import contextlib
        with contextlib.ExitStack() as ctx:
            consts = ctx.enter_context(tc.tile_pool(name="consts", bufs=1))
            dram = ctx.enter_context(tc.tile_pool(name="dram", bufs=1, space="DRAM"))

            # ---- constants ----
            ident = consts.tile([128, 128], BF16)
            make_identity(nc, ident)
            cmask = consts.tile([128, 128], F32)
            make_causal_mask(nc, cmask, mask_val=-1e10)
            eps_t = consts.tile([128, 1], F32)
            nc.vector.memset(eps_t, EPS)

            bq_s = consts.tile([D2, 1], F32, tag="bq")
            nc.sync.dma_start(out=bq_s, in_=bq_d)
            bk_s = consts.tile([D2, 1], F32, tag="bk")
            nc.sync.dma_start(out=bk_s, in_=bk_d)
            bv_s = consts.tile([128, D2], F32, tag="bv")
            nc.sync.dma_start(out=bv_s, in_=_bcast_ap(bv_d))
            bp_s = consts.tile([128, C], F32, tag="bp")
            nc.sync.dma_start(out=bp_s, in_=_bcast_ap(bp_d))
            b2_s = consts.tile([128, C], F32, tag="b2")
            nc.sync.dma_start(out=b2_s, in_=_bcast_ap(b2_d))
            g1_s = consts.tile([128, C], F32, tag="g1")
            nc.sync.dma_start(out=g1_s, in_=_bcast_ap(g1_d))
            be1_s = consts.tile([128, C], F32, tag="be1")
            nc.sync.dma_start(out=be1_s, in_=_bcast_ap(be1_d))
            g2_s = consts.tile([128, C], F32, tag="g2")
            nc.sync.dma_start(out=g2_s, in_=_bcast_ap(g2_d))
            be2_s = consts.tile([128, C], F32, tag="be2")
            nc.sync.dma_start(out=be2_s, in_=_bcast_ap(be2_d))
            b1_s = consts.tile([128, FFC, 1], F32, tag="b1")
            nc.sync.dma_start(out=b1_s, in_=b1_d.rearrange("(fh fl) o -> fl fh o", fl=128))

            # qkv weights: [C, D2] -> sbuf [128, CC, D2]
            wq_s = consts.tile([128, CC, D2], BF16, tag="wq")
            nc.sync.dma_start(out=wq_s, in_=wq_d.rearrange("(ch cl) d -> cl ch d", cl=128))
            wk_s = consts.tile([128, CC, D2], BF16, tag="wk")
            nc.sync.dma_start(out=wk_s, in_=wk_d.rearrange("(ch cl) d -> cl ch d", cl=128))
            wv_s = consts.tile([128, CC, D2], BF16, tag="wv")
            nc.sync.dma_start(out=wv_s, in_=wv_d.rearrange("(ch cl) d -> cl ch d", cl=128))
            wp_s = consts.tile([128, CC, C], BF16, tag="wp")
            nc.sync.dma_start(out=wp_s, in_=wp_d.rearrange("(ch cl) e -> cl ch e", cl=128))

            a2a_in = dram.tile([C, S], BF16, tag="a2a_in", name="a2a_in")
            a2a_out = dram.tile([C, S], BF16, tag="a2a_out", name="a2a_out")

            # ================= attention era =================
            with contextlib.ExitStack() as attn_ctx:
                apool = attn_ctx.enter_context(tc.tile_pool(name="attn", bufs=1))
                work = attn_ctx.enter_context(tc.tile_pool(name="workA", bufs=3))

                xT = [apool.tile([128, CC, T], BF16, tag=f"xT{b}", name=f"xT{b}") for b in range(B)]
                QT = [apool.tile([D2, T], BF16, tag=f"QT{b}", name=f"QT{b}") for b in range(B)]
                KT = [apool.tile([D2, T], BF16, tag=f"KT{b}", name=f"KT{b}") for b in range(B)]
                V = [apool.tile([128, NT, D2], BF16, tag=f"V{b}", name=f"V{b}") for b in range(B)]
                PTs = [apool.tile([128, S], BF16, tag=f"pt{sb}", name=f"pt{sb}")
                       for sb in range(NT)]

                # ---- phase A: load x, transpose to xT ----
                with tc.tile_pool(name="psA", bufs=1, space="PSUM") as psA:
                    for b in range(B):
                        for i in range(NT):
                            xt = work.tile([128, C], BF16, tag="x_in")
                            nc.sync.dma_start(out=xt, in_=x_d[b * T + i * 128: b * T + (i + 1) * 128, :])
                            for k in range(CC):
                                pt = psA.tile([128, 128], BF16, tag="tp", bufs=4)
                                nc.tensor.transpose(pt, xt[:, k * 128:(k + 1) * 128], ident)
                                nc.vector.tensor_copy(out=xT[b][:, k, i * 128:(i + 1) * 128], in_=pt)

                # ---- phase B: Q/K projections ([D2, T]) and V ([s, D2]) ----
                with tc.tile_pool(name="psB", bufs=1, space="PSUM") as psB:
                    for b in range(B):
                        for w_s, bias_s, dst in ((wq_s, bq_s, QT[b]), (wk_s, bk_s, KT[b])):
                            for j in range(T // SC):
                                ps = psB.tile([D2, SC], F32, tag="qk_ps", bufs=3)
                                for k in range(CC):
                                    nc.tensor.matmul(ps, w_s[:, k, :], xT[b][:, k, j * SC:(j + 1) * SC],
                                                     start=(k == 0), stop=(k == CC - 1))
                                nc.vector.tensor_scalar_add(out=dst[:, j * SC:(j + 1) * SC],
                                                            in0=ps, scalar1=bias_s)
                        for sb in range(NT):
                            ps = psB.tile([128, D2], F32, tag="v_ps", bufs=3)
                            for k in range(CC):
                                nc.tensor.matmul(ps, xT[b][:, k, sb * 128:(sb + 1) * 128], wv_s[:, k, :],
                                                 start=(k == 0), stop=(k == CC - 1))
                            nc.vector.tensor_tensor(out=V[b][:, sb, :], in0=ps, in1=bv_s,
                                                    op=mybir.AluOpType.add)

                # ---- phase C: attention per (batch, row-group, head) ----
                with tc.tile_pool(name="psC", bufs=1, space="PSUM") as psC:
                    for b in range(B):
                        for g in range(G_PB):
                            n_sb = (g + 1) * QG
                            outT_s = work.tile([D2, S], BF16, tag="outT", bufs=2)
                            for h in range(HPC):
                                hofs = h * HS
                                for ql in range(QG):
                                    qi = g * QG + ql
                                    s_len = (qi + 1) * 128
                                    n_ch = (s_len + SC - 1) // SC
                                    p_t = work.tile([128, NT * 128], BF16, tag="p", bufs=2)
                                    sums = work.tile([128, (NT * 128 + SC - 1) // SC], F32,
                                                     tag="sums", bufs=2)
                                    for ci in range(n_ch):
                                        w = min(SC, s_len - ci * SC)
                                        ps = psC.tile([128, SC], F32, tag="sc", bufs=2)
                                        nc.tensor.matmul(
                                            ps[:, :w],
                                            QT[b][hofs:hofs + HS, qi * 128:(qi + 1) * 128],
                                            KT[b][hofs:hofs + HS, ci * SC:ci * SC + w],
                                            start=True, stop=True)
                                        if (ci + 1) * SC >= s_len:  # chunk w/ diagonal block
                                            nc.vector.tensor_tensor(
                                                out=ps[:, w - 128:w], in0=ps[:, w - 128:w],
                                                in1=cmask, op=mybir.AluOpType.add)
                                        nc.scalar.activation(
                                            out=p_t[:, ci * SC:ci * SC + w], in_=ps[:, :w],
                                            func=mybir.ActivationFunctionType.Exp,
                                            scale=scale, accum_out=sums[:, ci:ci + 1])
                                    tot = work.tile([128, 1], F32, tag="tot", bufs=2)
                                    nc.vector.reduce_sum(tot, sums[:, :n_ch], mybir.AxisListType.X)
                                    rec = work.tile([128, 1], F32, tag="rec", bufs=2)
                                    nc.vector.reciprocal(rec, tot)
                                    nc.vector.tensor_scalar_mul(out=p_t[:, :s_len],
                                                                in0=p_t[:, :s_len], scalar1=rec)
                                    for sb in range(qi + 1):
                                        pt_ps = psC.tile([128, 128], BF16, tag="ptp", bufs=3)
                                        nc.tensor.transpose(pt_ps, p_t[:, sb * 128:(sb + 1) * 128], ident)
                                        nc.vector.tensor_copy(out=PTs[sb][:, ql * 128:(ql + 1) * 128],
                                                              in_=pt_ps)
                                # attnV for head h
                                avp = psC.tile([HS, S], F32, tag="av", bufs=2)
                                for sb in range(n_sb):
                                    sb_l = sb - g * QG
                                    col0 = 0 if sb_l < 0 else sb_l * 128
                                    nc.tensor.matmul(avp[:, col0:], V[b][:, sb, hofs:hofs + HS],
                                                     PTs[sb][:, col0:],
                                                     start=(sb == 0), stop=(sb == n_sb - 1))
                                nc.vector.tensor_copy(out=outT_s[hofs:hofs + HS, :], in_=avp)
                            j = b * G_PB + g
                            nc.sync.dma_start(out=a2a_in[j * D2:(j + 1) * D2, :], in_=outT_s)

            # ---- phase D: AllToAll ----
            nc.gpsimd.collective_compute(
                "AllToAll", mybir.AluOpType.bypass,
                replica_groups=[list(range(n_cores))],
                ins=[a2a_in.opt()], outs=[a2a_out.opt()])

            # ================= row-local era =================
            with contextlib.ExitStack() as post_ctx:
                ppool = post_ctx.enter_context(tc.tile_pool(name="post", bufs=1))
                work = post_ctx.enter_context(tc.tile_pool(name="workB", bufs=2))

                ot_s = ppool.tile([128, CC, S], BF16, tag="ot")
                nc.sync.dma_start(out=ot_s, in_=a2a_out.rearrange("(ch cl) s -> cl ch s", cl=128))
                hln = ppool.tile([128, S // 128, C], BF16, tag="hln")
                hlnT = ppool.tile([128, CC, S], BF16, tag="hlnT")
                fT = ppool.tile([128, FFC, S], BF16, tag="fT")

                # ---- phase E: proj + bp + LN1 ----
                with tc.tile_pool(name="psE", bufs=1, space="PSUM") as psE:
                    for ti in range(S // 128):
                        hps = []
                        for e in range(C // SC):
                            pp = psE.tile([128, SC], F32, tag=f"proj{e}", bufs=2, name=f"proj{e}")
                            for k in range(CC):
                                nc.tensor.matmul(pp, ot_s[:, k, ti * 128:(ti + 1) * 128],
                                                 wp_s[:, k, e * SC:(e + 1) * SC],
                                                 start=(k == 0), stop=(k == CC - 1))
                            hps.append(pp)
                        h_t = work.tile([128, C], F32, tag="h", bufs=2)
                        for e, pp in enumerate(hps):
                            nc.vector.tensor_tensor(out=h_t[:, e * SC:(e + 1) * SC], in0=pp,
                                                    in1=bp_s[:, e * SC:(e + 1) * SC],
                                                    op=mybir.AluOpType.add)
                        _ln(nc, work, h_t, g1_s, be1_s, hln[:, ti, :], eps_t, C)

                # ---- phase F: transpose hln -> hlnT [128, CC, S] ----
                with tc.tile_pool(name="psF", bufs=1, space="PSUM") as psF:
                    for ti in range(S // 128):
                        for k in range(CC):
                            pt = psF.tile([128, 128], BF16, tag="tp2", bufs=4)
                            nc.tensor.transpose(pt, hln[:, ti, k * 128:(k + 1) * 128], ident)
                            nc.vector.tensor_copy(out=hlnT[:, k, ti * 128:(ti + 1) * 128], in_=pt)

                # ---- phase G: FFN1 (fT = relu(W1.T @ hlnT + b1)) ----
                with tc.tile_pool(name="psG", bufs=1, space="PSUM") as psG:
                    for f in range(FFC):
                        w1_t = work.tile([128, CC, 128], BF16, tag="w1", bufs=3)
                        nc.sync.dma_start(out=w1_t, in_=w1_d[f].rearrange("(ch cl) f -> cl ch f", cl=128))
                        fp = psG.tile([128, S], F32, tag="f1", bufs=2)
                        for k in range(CC):
                            nc.tensor.matmul(fp, w1_t[:, k, :], hlnT[:, k, :],
                                             start=(k == 0), stop=(k == CC - 1))
                        nc.scalar.activation(out=fT[:, f, :], in_=fp,
                                             func=mybir.ActivationFunctionType.Relu,
                                             bias=b1_s[:, f, :])

                # ---- phase H: FFN2 (g = relu(fT.T @ W2 + b2)), + LN2 -> y ----
                with tc.tile_pool(name="psH", bufs=1, space="PSUM") as psH:
                    n_eh = C // SC
                    gps = [[psH.tile([128, SC], F32, tag=f"g{t}_{e}", bufs=1, name=f"g{t}_{e}")
                            for e in range(n_eh)] for t in range(S // 128)]
                    for f in range(FFC):
                        w2_t = work.tile([128, C], BF16, tag="w2", bufs=3)
                        nc.sync.dma_start(out=w2_t, in_=w2_d[f * 128:(f + 1) * 128, :])
                        for t in range(S // 128):
                            for e in range(n_eh):
                                nc.tensor.matmul(gps[t][e], fT[:, f, t * 128:(t + 1) * 128],
                                                 w2_t[:, e * SC:(e + 1) * SC],
                                                 start=(f == 0), stop=(f == FFC - 1))
                    for t in range(S // 128):
                        g_t = work.tile([128, C], F32, tag="gt", bufs=2)
                        for e in range(n_eh):
                            nc.vector.tensor_tensor(out=g_t[:, e * SC:(e + 1) * SC], in0=gps[t][e],
                                                    in1=b2_s[:, e * SC:(e + 1) * SC],
                                                    op=mybir.AluOpType.add)
                        nc.vector.tensor_scalar_max(out=g_t, in0=g_t, scalar1=0.0)
                        o_t = work.tile([128, C], F32, tag="ot_f", bufs=2)
                        _ln(nc, work, g_t, g2_s, be2_s, o_t, eps_t, C)
                        nc.sync.dma_start(out=y_d[t * 128:(t + 1) * 128, :], in_=o_t)

    nc.compile()
    cfg = dict(B=B, T=T, C=C, H=H, HS=HS, HPC=HPC, D2=D2, S=S, FF=FF,
               n_cores=n_cores, G_PB=G_PB)
    return nc, cfg


def _ln(nc, pool, x_t, gamma_s, beta_s, out_ap, eps_t, C):
    """LayerNorm rows of x_t [128, C] f32 -> out_ap (any dtype)."""
    ng = (C + 511) // 512
    stats = pool.tile([128, ng, 6], F32, tag="ln_stats", bufs=2)
    for i in range(ng):
        w = min(512, C - i * 512)
        nc.vector.bn_stats(out=stats[:, i, :], in_=x_t[:, i * 512:i * 512 + w])
    mv = pool.tile([128, 2], F32, tag="ln_mv", bufs=2)
    nc.vector.bn_aggr(out=mv, in_=stats)
    rstd = pool.tile([128, 1], F32, tag="ln_rstd", bufs=2)
    nc.scalar.activation(out=rstd, in_=mv[:, 1:2],
                         func=mybir.ActivationFunctionType.Sqrt, bias=eps_t)
    nc.vector.reciprocal(rstd, rstd)
    tmp = pool.tile([128, C], F32, tag="ln_tmp", bufs=2)
    nc.vector.tensor_scalar(out=tmp, in0=x_t, scalar1=mv[:, 0:1], scalar2=rstd,
                            op0=mybir.AluOpType.subtract, op1=mybir.AluOpType.mult)
    nc.vector.tensor_tensor(out=tmp, in0=tmp, in1=gamma_s, op=mybir.AluOpType.mult)
    nc.vector.tensor_tensor(out=out_ap, in0=tmp, in1=beta_s, op=mybir.AluOpType.add)


# ============================================================================
# Host side: shard inputs, run SPMD, unshard output.
# ============================================================================

def make_in_maps(inputs, cfg):
    """Build the per-core input dicts from full-size numpy inputs."""
    B, T, C, H = cfg["B"], cfg["T"], cfg["C"], cfg["H"]
    HS, HPC, FF, n_cores = cfg["HS"], cfg["HPC"], cfg["FF"], cfg["n_cores"]
    FFC = FF // 128
    bf = ml_dtypes.bfloat16

    f = {k: np.asarray(v, np.float32) for k, v in inputs.items()}
    x_flat = f["x"].reshape(B * T, C).astype(bf)
    w1t = f["W1"].reshape(C, FFC, 128).transpose(1, 0, 2).astype(bf)
    w2 = f["W2"].astype(bf)
    wp = f["Wp"].astype(bf)
    shared = dict(
        x=x_flat, wp=wp, bp=f["bp"], w1t=w1t, b1=f["b1"].reshape(FF, 1),
        w2=w2, b2=f["b2"], g1=f["g1"], be1=f["be1"], g2=f["g2"], be2=f["be2"],
    )
    in_maps = []
    for c in range(n_cores):
        hs = slice(c * HPC, (c + 1) * HPC)
        # [HPC, C, HS] -> [C, HPC*HS]
        wq = f["Wq"][hs].transpose(1, 0, 2).reshape(C, HPC * HS).astype(bf)
        wk = f["Wk"][hs].transpose(1, 0, 2).reshape(C, HPC * HS).astype(bf)
        wv = f["Wv"][hs].transpose(1, 0, 2).reshape(C, HPC * HS).astype(bf)
        bq = f["bq"][hs].reshape(HPC * HS, 1)
        bk = f["bk"][hs].reshape(HPC * HS, 1)
        bv = f["bv"][hs].reshape(HPC * HS)
        in_maps.append(dict(shared, wq=wq, wk=wk, wv=wv, bq=bq, bk=bk, bv=bv))
    return in_maps


def unshard(results, cfg):
    B, T, C, S = cfg["B"], cfg["T"], cfg["C"], cfg["S"]
    out = np.empty((B * T, C), np.float32)
    for c, r in enumerate(results):
        out[c * S:(c + 1) * S, :] = r["y"]
    return out.reshape(B, T, C)


_BUILT = None


def _get_built():
    global _BUILT
    if _BUILT is None:
        _BUILT = build()
    return _BUILT


def _install_ntff_hook():
    """Best-effort: register the axon NTFF profiling hook (for trace=True)."""
    try:
        import antenv
        if "antenv.axon_hooks" not in sys.modules:
            m = types.ModuleType("antenv.axon_hooks")
            h = [None]
            m.set_axon_ntff_profile_hook = lambda x: h.__setitem__(0, x)
            m.get_axon_ntff_profile_hook = lambda: h[0]
            sys.modules["antenv.axon_hooks"] = m
            antenv.axon_hooks = m
            from trn_agent_boot.trn_boot import _ntff_profile_via_ctypes
            m.set_axon_ntff_profile_hook(
                _ntff_profile_via_ctypes("/opt/axon/libaxon_pjrt.so"))
    except Exception:
        pass


def run(inputs, trace=False):
    _install_ntff_hook()
    from concourse import bass_utils
    bass_utils.upload_artifacts = lambda tmpdir: "local://" + tmpdir
    nc, cfg = _get_built()
    in_maps = make_in_maps(inputs, cfg)
    res = bass_utils.run_bass_kernel_spmd(
        nc, in_maps, list(range(cfg["n_cores"])), trace=trace)
    return unshard(res.results, cfg), res


def kernel(**inputs):
    out, _ = run(inputs, trace=False)
    return out
